# revision 1
# baseline (speedup 1.0000x reference)
"""Trainium2 Bass kernel for LoRALayer: out = 2.0 * (x @ B) @ A.

x: [4, 4096, 4096] f32; A: [8, 4096] f32; B: [4096, 8] f32.
Sharding: data-parallel on the 16384 tokens across 8 cores (2048 each);
A/B replicated. Host-side prep (part of sharding): each core's x-shard is
shipped transposed (contraction dim on SBUF partitions) and split into
bf16 hi/lo halves (x = hi + lo exactly captures 16 mantissa bits); B and
2*A likewise. bf16 matmuls are ~4x cheaper than fp32 on the PE (single
pass + fast weight load), and the hi/lo compensation keeps ~1e-5 accuracy.

Per core, per TBLK-token block (f32 PSUM accumulation; 32-aligned row blocks
because engine partition bases must be multiples of 32):
  mm1 chain A: ps_a[40,TBLK] += [B_hi|0|B_lo]_c.T @ xh_c  (32 chunks; M-packed:
               rows 0-7 = x_hi@B_hi, rows 32-39 = x_hi@B_lo)
  mm1 chain B: ps_b[8,TBLK]  += B_hi_c.T @ xl_c           (x_lo@B_hi)
  y = ps_a[0:8] + ps_a[32:40] + ps_b  (DVE, f32); split y -> y_hi/y_lo bf16,
  pack K-wise as rows {0-7: y_hi, 32-39: y_hi, 64-71: y_lo} against
  A2pk rows {0-7: A_hi, 32-39: A_lo, 64-71: A_hi} (zeros elsewhere):
  mm2: out[128,512] = y_pack_sub[96,128].T @ A2pk[96,512]  (one MM = all 3
       correction terms; zero rows contribute nothing).
mm2 of block b-1 is interleaved with mm1 of block b (PE density for HAM);
input DMAs ride the sync HWDGE ring, output DMAs the scalar ring; PSUM->SBUF
copies alternate DVE/ACT.
"""

import numpy as np

P = 128
F_IN = 4096
F_OUT = 4096
RANK = 8
N_CORES = 8
SCALING = 2.0
TBLK = 256             # token block (mm1 rhs free dim, max 512 for f32 PSUM out)

_CACHE = {}


def _build_nc(T, F_in, F_out, R):
    """Build the single-core Bass program for a T-token shard."""
    from contextlib import ExitStack

    import concourse.mybir as mybir
    import concourse.tile as tile
    from concourse import bacc

    f32 = mybir.dt.float32
    bf16 = mybir.dt.bfloat16
    tblk = min(TBLK, T)
    CH = F_in // P          # feature chunks (32)
    NB = T // tblk          # token blocks (4)
    NSUB = tblk // P        # 128-token subtiles per block (4)
    NS = F_out // 512       # output column chunks (8)
    CGRP = min(8, CH)       # chunks per input sub-DMA (1MB bf16 granularity)
    NDMA = CH // CGRP       # input sub-DMAs per tensor per block (4)
    MM2_PER_BLK = NSUB * NS  # 32
    RB = 32                  # 32-aligned row blocks (engine partition bases)

    nc = bacc.Bacc("TRN2", target_bir_lowering=False, debug=False)

    xh_d = nc.dram_tensor(
        "xh", [NB, NDMA, P, CGRP * tblk], bf16, kind="ExternalInput"
    ).ap()
    xl_d = nc.dram_tensor(
        "xl", [NB, NDMA, P, CGRP * tblk], bf16, kind="ExternalInput"
    ).ap()
    bpk_d = nc.dram_tensor("Bpk", [P, CH * 2 * RB], bf16, kind="ExternalInput").ap()
    a2pk_d = nc.dram_tensor("A2pk", [3 * RB, F_out], bf16, kind="ExternalInput").ap()
    out_d = nc.dram_tensor("out", [T, F_out], f32, kind="ExternalOutput").ap()

    with tile.TileContext(nc) as tc, ExitStack() as ctx:
        cpool = ctx.enter_context(tc.tile_pool(name="const", bufs=1))
        xtpool = ctx.enter_context(tc.tile_pool(name="xt", bufs=2 * NDMA))
        ytpool = ctx.enter_context(tc.tile_pool(name="yt", bufs=3))
        opool = ctx.enter_context(tc.tile_pool(name="osb", bufs=3))
        y_pp = ctx.enter_context(tc.tile_pool(name="y_ps", bufs=2, space="PSUM"))
        o_pp = ctx.enter_context(tc.tile_pool(name="o_ps", bufs=4, space="PSUM"))

        bpk_sb = cpool.tile([P, CH * 2 * RB], bf16, tag="bpk_sb")
        nc.sync.dma_start(bpk_sb[:], bpk_d)
        apk_sb = cpool.tile([3 * RB, F_out], bf16, tag="apk_sb")
        nc.sync.dma_start(apk_sb[:], a2pk_d)

        blk_state = {}

        def emit_mm2(blk, idx):
            """idx in [0, MM2_PER_BLK): (sub, n) pair for block blk."""
            sub, n = divmod(idx, NS)
            y_pack, o_sbs = blk_state[blk]
            if n == 0:
                o_sbs[sub] = opool.tile(
                    [P, F_out], f32, tag="o_sb", name=f"o_sb_{blk}_{sub}"
                )
            o_sb = o_sbs[sub]
            o_ps = o_pp.tile([P, 512], f32, tag="o_ps")
            nc.tensor.matmul(
                o_ps[:],
                y_pack[:, sub * P:(sub + 1) * P],
                apk_sb[:, n * 512:(n + 1) * 512],
                start=True,
                stop=True,
            )
            if n % 2 == 0:
                nc.scalar.copy(o_sb[:, n * 512:(n + 1) * 512], o_ps[:])
            else:
                nc.vector.tensor_copy(o_sb[:, n * 512:(n + 1) * 512], o_ps[:])
            if n == NS - 1:
                trow = blk * tblk + sub * P
                nc.scalar.dma_start(out_d[trow:trow + P, :], o_sb[:])

        for blk in range(NB + 1):
            xhs, xls = [], []
            if blk < NB:
                for s in range(NDMA):
                    xh_sb = xtpool.tile([P, CGRP, tblk], bf16, tag="xh_sb")
                    nc.sync.dma_start(
                        xh_sb[:].rearrange("p c t -> p (c t)"), xh_d[blk, s]
                    )
                    xhs.append(xh_sb)
                    xl_sb = xtpool.tile([P, CGRP, tblk], bf16, tag="xl_sb")
                    nc.sync.dma_start(
                        xl_sb[:].rearrange("p c t -> p (c t)"), xl_d[blk, s]
                    )
                    xls.append(xl_sb)
                ps_a = y_pp.tile([RB + R, tblk], f32, tag="ps_a")
                ps_b = y_pp.tile([R, tblk], f32, tag="ps_b")

            # Interleave mm1 of this block 1:1 with mm2 of the previous block.
            n_steps = max(CH if blk < NB else 0, MM2_PER_BLK if blk > 0 else 0)
            for i in range(n_steps):
                if blk > 0 and i < MM2_PER_BLK:
                    emit_mm2(blk - 1, i)
                if blk < NB and i < CH:
                    c = i
                    nc.tensor.matmul(
                        ps_a[:],
                        bpk_sb[:, c * 2 * RB:c * 2 * RB + RB + R],
                        xhs[c // CGRP][:, c % CGRP, :],
                        start=(c == 0),
                        stop=(c == CH - 1),
                    )
                    nc.tensor.matmul(
                        ps_b[:],
                        bpk_sb[:, c * 2 * RB:c * 2 * RB + R],
                        xls[c // CGRP][:, c % CGRP, :],
                        start=(c == 0),
                        stop=(c == CH - 1),
                    )
            if blk > 0:
                del blk_state[blk - 1]
            if blk < NB:
                # y = hh + hl + lh (f32), then split into bf16 hi/lo and pack
                # K-wise as [y_hi; y_hi; y_lo] for the one-shot mm2.
                yt32 = ytpool.tile([R, tblk], f32, tag="yt32")
                nc.vector.tensor_copy(yt32[:], ps_a[:R, :])
                nc.vector.tensor_add(yt32[:], yt32[:], ps_a[RB:RB + R, :])
                nc.vector.tensor_add(yt32[:], yt32[:], ps_b[:])
                y_pack = ytpool.tile([3 * RB, tblk], bf16, tag="y_pack")
                nc.gpsimd.memset(y_pack[:], 0.0)
                nc.vector.tensor_copy(y_pack[:R, :], yt32[:])               # y_hi
                nc.vector.tensor_copy(y_pack[RB:RB + R, :], y_pack[:R, :])  # dup
                y_hi32 = ytpool.tile([R, tblk], f32, tag="y_hi32")
                nc.vector.tensor_copy(y_hi32[:], y_pack[:R, :])             # f32
                nc.vector.tensor_sub(y_pack[2 * RB:2 * RB + R, :], yt32[:], y_hi32[:])
                blk_state[blk] = (y_pack, {})

    nc.compile()
    return nc


def _pack_inputs(x2d, A, B, T_shard, F_in, R):
    """Shard x on tokens (transposed + bf16 hi/lo split); replicate B/A packs."""
    import ml_dtypes

    bf16 = ml_dtypes.bfloat16
    CH = F_in // P

    def split(m):
        hi = m.astype(bf16)
        lo = (m - hi.astype(np.float32)).astype(bf16)
        return hi, lo

    RB = 32
    R = B.shape[1]
    Bh, Bl = split(B.astype(np.float32))
    # chunk-major pack, 32-aligned: per chunk c of 2*RB cols:
    #   [0:R]=B_hi, [RB:RB+R]=B_lo, rest zero
    bpk = np.zeros((CH, P, 2 * RB), dtype=Bh.dtype)
    bpk[:, :, :R] = Bh.reshape(CH, P, R)
    bpk[:, :, RB:RB + R] = Bl.reshape(CH, P, R)
    bpk = np.ascontiguousarray(bpk.transpose(1, 0, 2).reshape(P, CH * 2 * RB))

    A2 = (SCALING * A).astype(np.float32)
    Ah, Al = split(A2)
    a2pk = np.zeros((3 * RB, A2.shape[1]), dtype=Ah.dtype)
    a2pk[:R] = Ah
    a2pk[RB:RB + R] = Al
    a2pk[2 * RB:2 * RB + R] = Ah
    a2pk = np.ascontiguousarray(a2pk)

    # device-DMA-friendly pack: [NB, NDMA, P, CGRP*tblk] so each sub-DMA
    # reads one contiguous per-partition run.
    T = T_shard
    tblk = min(TBLK, T)
    NB = T // tblk
    CGRP = min(8, CH)
    NDMA = CH // CGRP

    def pack(m):
        a = m.reshape(NDMA, CGRP, P, NB, tblk)
        a = a.transpose(3, 0, 2, 1, 4)
        return np.ascontiguousarray(a.reshape(NB, NDMA, P, CGRP * tblk))

    n_shards = x2d.shape[0] // T_shard
    in_maps = []
    for c in range(n_shards):
        xt = np.ascontiguousarray(x2d[c * T_shard:(c + 1) * T_shard].T)
        xh, xl = split(xt)
        in_maps.append(
            {"xh": pack(xh), "xl": pack(xl), "Bpk": bpk, "A2pk": a2pk}
        )
    return in_maps


def kernel(x, A, B):
    from concourse.bass_utils import run_bass_kernel_spmd

    x = np.asarray(x, dtype=np.float32)
    A = np.asarray(A, dtype=np.float32)
    B = np.asarray(B, dtype=np.float32)
    orig_shape = x.shape
    x2d = x.reshape(-1, F_IN)
    T_shard = x2d.shape[0] // N_CORES

    key = (T_shard, F_IN, F_OUT, RANK)
    if key not in _CACHE:
        _CACHE[key] = _build_nc(T_shard, F_IN, F_OUT, RANK)
    nc = _CACHE[key]

    in_maps = _pack_inputs(x2d, A, B, T_shard, F_IN, RANK)
    res = run_bass_kernel_spmd(nc, in_maps, core_ids=list(range(N_CORES)))
    out = np.concatenate([r["out"] for r in res.results], axis=0)
    return out.reshape(*orig_shape[:-1], F_OUT)



# revision 2
# speedup vs baseline: 1.6306x; 1.6306x over previous
"""Trainium2 Bass kernel for LoRALayer: out = 2.0 * (x @ B) @ A.

x: [4, 4096, 4096] f32; A: [8, 4096] f32; B: [4096, 8] f32.
Sharding: data-parallel on the 16384 tokens across 8 cores (2048 each);
A/B replicated. Host-side prep (part of sharding): each core's x-shard is
shipped transposed (contraction dim on SBUF partitions) as plain bf16;
B and 2*A likewise. The kernel is HBM-bandwidth bound (~358 GB/s/core),
so bytes moved are the whole game: bf16 x in (16.8 MB/core) + bf16 out
(16.8 MB/core, upcast to f32 on host) ≈ 33.6 MB/core ≈ 94 us roofline.
All-bf16 numerics land at ~7e-3 absmax-rel vs the f32 reference, well
inside the 2e-2 gate (f32 PSUM accumulation throughout).

Per core, per 512-token block (f32 PSUM accumulation):
  mm1: ps_y[8,512] += B_c.T @ xt_c  over 32 feature chunks (K=128 each)
  y_sb = bf16(ps_y)  (DVE)
  mm2: for each 128-token subtile and 512-col chunk:
       o_ps[128,512] = y_sb[:,sub].T @ A2[:,n]  (K=8, one shot)
       o_sb[128, n*512:] = bf16(o_ps)  (alternate ACT/DVE)
mm2 of block b-1 is interleaved 1:1 with mm1 of block b (PE stays dense
for the HAM clock gate); input DMAs ride the sync HWDGE ring, output
DMAs the scalar ring.
"""

import numpy as np

P = 128
F_IN = 4096
F_OUT = 4096
RANK = 8
N_CORES = 8
SCALING = 2.0
TBLK = 512             # token block (mm1 rhs free dim = one f32 PSUM bank)

_CACHE = {}


def _build_nc(T, F_in, F_out, R):
    """Build the single-core Bass program for a T-token shard."""
    from contextlib import ExitStack

    import concourse.mybir as mybir
    import concourse.tile as tile
    from concourse import bacc

    f32 = mybir.dt.float32
    bf16 = mybir.dt.bfloat16
    tblk = min(TBLK, T)
    CH = F_in // P          # feature chunks (32)
    NB = T // tblk          # token blocks (4)
    NSUB = tblk // P        # 128-token subtiles per block (4)
    NS = F_out // 512       # output column chunks (8)
    CGRP = min(8, CH)       # chunks per input sub-DMA
    NDMA = CH // CGRP       # input sub-DMAs per block (4)
    MM2_PER_BLK = NSUB * NS  # 32

    nc = bacc.Bacc("TRN2", target_bir_lowering=False, debug=False)

    xt_d = nc.dram_tensor(
        "xt", [NB, NDMA, P, CGRP * tblk], bf16, kind="ExternalInput"
    ).ap()
    bpk_d = nc.dram_tensor("Bpk", [P, CH * R], bf16, kind="ExternalInput").ap()
    a2_d = nc.dram_tensor("A2", [R, F_out], bf16, kind="ExternalInput").ap()
    out_d = nc.dram_tensor("out", [T, F_out], bf16, kind="ExternalOutput").ap()

    with tile.TileContext(nc) as tc, ExitStack() as ctx:
        cpool = ctx.enter_context(tc.tile_pool(name="const", bufs=1))
        xtpool = ctx.enter_context(tc.tile_pool(name="xt", bufs=2 * NDMA))
        ytpool = ctx.enter_context(tc.tile_pool(name="yt", bufs=2))
        opool = ctx.enter_context(tc.tile_pool(name="osb", bufs=3))
        y_pp = ctx.enter_context(tc.tile_pool(name="y_ps", bufs=2, space="PSUM"))
        o_pp = ctx.enter_context(tc.tile_pool(name="o_ps", bufs=4, space="PSUM"))

        bpk_sb = cpool.tile([P, CH * R], bf16, tag="bpk_sb")
        nc.sync.dma_start(bpk_sb[:], bpk_d)
        a2_sb = cpool.tile([R, F_out], bf16, tag="a2_sb")
        nc.sync.dma_start(a2_sb[:], a2_d)

        blk_state = {}

        def emit_mm2(blk, idx):
            """idx in [0, MM2_PER_BLK): (sub, n) pair for block blk."""
            sub, n = divmod(idx, NS)
            y_sb, o_sbs = blk_state[blk]
            if n == 0:
                o_sbs[sub] = opool.tile(
                    [P, F_out], bf16, tag="o_sb", name=f"o_sb_{blk}_{sub}"
                )
            o_sb = o_sbs[sub]
            o_ps = o_pp.tile([P, 512], f32, tag="o_ps")
            nc.tensor.matmul(
                o_ps[:],
                y_sb[:, sub * P:(sub + 1) * P],
                a2_sb[:, n * 512:(n + 1) * 512],
                start=True,
                stop=True,
            )
            if n % 2 == 0:
                nc.scalar.copy(o_sb[:, n * 512:(n + 1) * 512], o_ps[:])
            else:
                nc.vector.tensor_copy(o_sb[:, n * 512:(n + 1) * 512], o_ps[:])
            if n == NS - 1:
                trow = blk * tblk + sub * P
                nc.scalar.dma_start(out_d[trow:trow + P, :], o_sb[:])

        for blk in range(NB + 1):
            xts = []
            if blk < NB:
                for s in range(NDMA):
                    xt_sb = xtpool.tile([P, CGRP, tblk], bf16, tag="xt_sb")
                    nc.sync.dma_start(
                        xt_sb[:].rearrange("p c t -> p (c t)"), xt_d[blk, s]
                    )
                    xts.append(xt_sb)
                ps_y = y_pp.tile([R, tblk], f32, tag="ps_y")

            # Interleave mm1 of this block 1:1 with mm2 of the previous block.
            n_steps = max(CH if blk < NB else 0, MM2_PER_BLK if blk > 0 else 0)
            for i in range(n_steps):
                if blk > 0 and i < MM2_PER_BLK:
                    emit_mm2(blk - 1, i)
                if blk < NB and i < CH:
                    c = i
                    nc.tensor.matmul(
                        ps_y[:],
                        bpk_sb[:, c * R:(c + 1) * R],
                        xts[c // CGRP][:, c % CGRP, :],
                        start=(c == 0),
                        stop=(c == CH - 1),
                    )
            if blk > 0:
                del blk_state[blk - 1]
            if blk < NB:
                y_sb = ytpool.tile([R, tblk], bf16, tag="y_sb")
                nc.vector.tensor_copy(y_sb[:], ps_y[:])
                blk_state[blk] = (y_sb, {})

    nc.compile()
    return nc


def _pack_inputs(x2d, A, B, T_shard, F_in, R):
    """Shard x on tokens (transposed, bf16); replicate bf16 B/2A packs."""
    import ml_dtypes

    bf16 = ml_dtypes.bfloat16
    CH = F_in // P

    # chunk-major B pack: col block c holds B chunk c ([128, R])
    bpk = np.ascontiguousarray(
        B.astype(np.float32).astype(bf16).reshape(CH, P, R)
        .transpose(1, 0, 2).reshape(P, CH * R)
    )
    a2 = np.ascontiguousarray((SCALING * A).astype(np.float32).astype(bf16))

    # device-DMA-friendly pack: [NB, NDMA, P, CGRP*tblk] so each sub-DMA
    # reads one contiguous per-partition run.
    T = T_shard
    tblk = min(TBLK, T)
    NB = T // tblk
    CGRP = min(8, CH)
    NDMA = CH // CGRP

    def pack(m):
        a = m.reshape(NDMA, CGRP, P, NB, tblk)
        a = a.transpose(3, 0, 2, 1, 4)
        return np.ascontiguousarray(a.reshape(NB, NDMA, P, CGRP * tblk))

    n_shards = x2d.shape[0] // T_shard
    in_maps = []
    for c in range(n_shards):
        xt = x2d[c * T_shard:(c + 1) * T_shard].T.astype(bf16)
        in_maps.append({"xt": pack(xt), "Bpk": bpk, "A2": a2})
    return in_maps


def kernel(x, A, B):
    from concourse.bass_utils import run_bass_kernel_spmd

    x = np.asarray(x, dtype=np.float32)
    A = np.asarray(A, dtype=np.float32)
    B = np.asarray(B, dtype=np.float32)
    orig_shape = x.shape
    x2d = x.reshape(-1, F_IN)
    T_shard = x2d.shape[0] // N_CORES

    key = (T_shard, F_IN, F_OUT, RANK)
    if key not in _CACHE:
        _CACHE[key] = _build_nc(T_shard, F_IN, F_OUT, RANK)
    nc = _CACHE[key]

    in_maps = _pack_inputs(x2d, A, B, T_shard, F_IN, RANK)
    res = run_bass_kernel_spmd(nc, in_maps, core_ids=list(range(N_CORES)))
    out = np.concatenate(
        [np.asarray(r["out"], dtype=np.float32) for r in res.results], axis=0
    )
    return out.reshape(*orig_shape[:-1], F_OUT)


# revision 4
# speedup vs baseline: 1.8382x; 1.1273x over previous
"""Trainium2 Bass kernel for LoRALayer: out = 2.0 * (x @ B) @ A.

x: [4, 4096, 4096] f32; A: [8, 4096] f32; B: [4096, 8] f32.
Sharding: data-parallel on the 16384 tokens across 8 cores (2048 each);
A/B replicated. Host-side prep (part of sharding): each core's x-shard is
shipped transposed (contraction dim on SBUF partitions) as plain bf16;
B and 2*A likewise. All-bf16 numerics land at ~7e-3 absmax-rel vs the
f32 reference (f32 PSUM accumulation), inside the 2e-2 gate. Output is
written bf16 and upcast to f32 on the host.

The PE is the bottleneck on this part (power governor holds the PE near
1.2 GHz under sustained load): mm1 must ingest x at 256 B/cycle and mm2
must emit out at 128 elem/cycle -> 131072 PE cycles/core ~ 109 us.
DMA (33.6 MB/core at ~358 GB/s) is ~94 us and hides under it. So the
schedule aims to keep the PE stream dense: fine-grained first-block
input DMAs (256 KB) so mm1 starts ASAP, mm2 of block b-1 interleaved
1:1 with mm1 of block b, 2-bank PSUM output tiles with [128,1024]
PSUM->SBUF copies alternating DVE/ACT, and half-row (512 KB) output
DMAs issued as soon as each half of an output subtile is ready.

Per core, per 512-token block (f32 PSUM accumulation):
  mm1: ps_y[8,512] += B_c.T @ xt_c  over 32 feature chunks (K=128 each)
  y_sb = bf16(ps_y)  (DVE)
  mm2: per 128-token subtile, 8 chunks: o_ps[128,512] = y_sb.T @ A2[:,n]
       copy pairs [128,1024] f32->bf16, DMA halves [128,2048] bf16.
"""

import numpy as np

P = 128
F_IN = 4096
F_OUT = 4096
RANK = 8
N_CORES = 8
SCALING = 2.0
TBLK = 512             # token block (mm1 rhs free dim = one f32 PSUM bank)
CGRP0 = 2              # chunks per input sub-DMA, first block (256 KB)
CGRP = 8               # chunks per input sub-DMA, later blocks (1 MB)

_CACHE = {}


def _build_nc(T, F_in, F_out, R):
    """Build the single-core Bass program for a T-token shard."""
    from contextlib import ExitStack

    import concourse.mybir as mybir
    import concourse.tile as tile
    from concourse import bacc

    f32 = mybir.dt.float32
    bf16 = mybir.dt.bfloat16
    tblk = min(TBLK, T)
    CH = F_in // P          # feature chunks (32)
    NB = T // tblk          # token blocks (4)
    NSUB = tblk // P        # 128-token subtiles per block (4)
    NS = F_out // 512       # output column chunks (8)
    NDMA0 = CH // CGRP0     # first-block input sub-DMAs (16)
    NDMA = CH // CGRP       # later-block input sub-DMAs (4)
    MM2_PER_BLK = NSUB * NS  # 32

    nc = bacc.Bacc("TRN2", target_bir_lowering=False, debug=False)

    xt0_d = nc.dram_tensor(
        "xt0", [NDMA0, P, CGRP0 * tblk], bf16, kind="ExternalInput"
    ).ap()
    xtr_d = nc.dram_tensor(
        "xtr", [max(NB - 1, 1), NDMA, P, CGRP * tblk], bf16, kind="ExternalInput"
    ).ap()
    bpk_d = nc.dram_tensor("Bpk", [P, CH * R], bf16, kind="ExternalInput").ap()
    a2_d = nc.dram_tensor("A2", [R, F_out], bf16, kind="ExternalInput").ap()
    out_d = nc.dram_tensor("out", [T, F_out], bf16, kind="ExternalOutput").ap()

    with tile.TileContext(nc) as tc, ExitStack() as ctx:
        cpool = ctx.enter_context(tc.tile_pool(name="const", bufs=1))
        xt0pool = ctx.enter_context(tc.tile_pool(name="xt0", bufs=NDMA0))
        xtpool = ctx.enter_context(tc.tile_pool(name="xt", bufs=2 * NDMA))
        ytpool = ctx.enter_context(tc.tile_pool(name="yt", bufs=2))
        opool = ctx.enter_context(tc.tile_pool(name="osb", bufs=4))
        y_pp = ctx.enter_context(tc.tile_pool(name="y_ps", bufs=2, space="PSUM"))
        o_pp = ctx.enter_context(tc.tile_pool(name="o_ps", bufs=3, space="PSUM"))

        bpk_sb = cpool.tile([P, CH * R], bf16, tag="bpk_sb")
        nc.sync.dma_start(bpk_sb[:], bpk_d)
        a2_sb = cpool.tile([R, F_out], bf16, tag="a2_sb")
        nc.sync.dma_start(a2_sb[:], a2_d)

        blk_state = {}

        def emit_mm2(blk, idx):
            """idx in [0, MM2_PER_BLK): (sub, n) pair for block blk."""
            sub, n = divmod(idx, NS)
            y_sb, o_sbs, o_pss = blk_state[blk]
            if n == 0:
                o_sbs[sub] = opool.tile(
                    [P, F_out], bf16, tag="o_sb", name=f"o_sb_{blk}_{sub}"
                )
            o_sb = o_sbs[sub]
            if n % 2 == 0:
                o_pss[sub] = o_pp.tile(
                    [P, 1024], f32, tag="o_ps", name=f"o_ps_{blk}_{sub}_{n}"
                )
            o_ps = o_pss[sub]
            nc.tensor.matmul(
                o_ps[:, (n % 2) * 512:(n % 2) * 512 + 512],
                y_sb[:, sub * P:(sub + 1) * P],
                a2_sb[:, n * 512:(n + 1) * 512],
                start=True,
                stop=True,
            )
            if n % 2 == 1:
                dst = o_sb[:, (n - 1) * 512:(n + 1) * 512]
                if (n // 2) % 2 == 0:
                    nc.vector.tensor_copy(dst, o_ps[:])
                else:
                    nc.scalar.copy(dst, o_ps[:])
            if n == NS // 2 - 1 or n == NS - 1:
                # DMA each half-row as soon as its copies are done.
                trow = blk * tblk + sub * P
                half = 0 if n == NS // 2 - 1 else 1
                cols = slice(half * (F_out // 2), (half + 1) * (F_out // 2))
                nc.scalar.dma_start(out_d[trow:trow + P, cols], o_sb[:, cols])

        for blk in range(NB + 1):
            xts = []
            cgrp = CGRP0 if blk == 0 else CGRP
            if blk < NB:
                for s in range(CH // cgrp):
                    pool = xt0pool if blk == 0 else xtpool
                    xt_sb = pool.tile(
                        [P, cgrp, tblk], bf16,
                        tag="xt0_sb" if blk == 0 else "xt_sb",
                    )
                    src = xt0_d[s] if blk == 0 else xtr_d[blk - 1, s]
                    nc.sync.dma_start(
                        xt_sb[:].rearrange("p c t -> p (c t)"), src
                    )
                    xts.append(xt_sb)
                ps_y = y_pp.tile([R, tblk], f32, tag="ps_y")

            # Interleave mm1 of this block 1:1 with mm2 of the previous block.
            n_steps = max(CH if blk < NB else 0, MM2_PER_BLK if blk > 0 else 0)
            for i in range(n_steps):
                if blk > 0 and i < MM2_PER_BLK:
                    emit_mm2(blk - 1, i)
                if blk < NB and i < CH:
                    c = i
                    nc.tensor.matmul(
                        ps_y[:],
                        bpk_sb[:, c * R:(c + 1) * R],
                        xts[c // cgrp][:, c % cgrp, :],
                        start=(c == 0),
                        stop=(c == CH - 1),
                    )
            if blk > 0:
                del blk_state[blk - 1]
            if blk < NB:
                y_sb = ytpool.tile([R, tblk], bf16, tag="y_sb")
                nc.vector.tensor_copy(y_sb[:], ps_y[:])
                blk_state[blk] = (y_sb, {}, {})

    nc.compile()
    return nc


def _pack_inputs(x2d, A, B, T_shard, F_in, R):
    """Shard x on tokens (transposed, bf16); replicate bf16 B/2A packs."""
    import ml_dtypes

    bf16 = ml_dtypes.bfloat16
    CH = F_in // P

    # chunk-major B pack: col block c holds B chunk c ([128, R])
    bpk = np.ascontiguousarray(
        B.astype(np.float32).astype(bf16).reshape(CH, P, R)
        .transpose(1, 0, 2).reshape(P, CH * R)
    )
    a2 = np.ascontiguousarray((SCALING * A).astype(np.float32).astype(bf16))

    # device-DMA-friendly packs: per sub-DMA one contiguous per-partition run.
    T = T_shard
    tblk = min(TBLK, T)
    NB = T // tblk

    def pack(m, nb, cgrp):
        ndma = CH // cgrp
        a = m.reshape(ndma, cgrp, P, nb, tblk)
        a = a.transpose(3, 0, 2, 1, 4)
        return np.ascontiguousarray(a.reshape(nb, ndma, P, cgrp * tblk))

    n_shards = x2d.shape[0] // T_shard
    in_maps = []
    for c in range(n_shards):
        xt = x2d[c * T_shard:(c + 1) * T_shard].T.astype(bf16)
        xt3 = xt.reshape(F_in, NB, tblk)
        xt0 = pack(np.ascontiguousarray(xt3[:, 0]), 1, CGRP0)[0]
        if NB > 1:
            xtr = pack(
                np.ascontiguousarray(
                    xt3[:, 1:].transpose(0, 1, 2).reshape(F_in, (NB - 1) * tblk)
                ),
                NB - 1, CGRP,
            )
        else:
            xtr = np.zeros((1, CH // CGRP, P, CGRP * tblk), dtype=bf16)
        in_maps.append({"xt0": xt0, "xtr": xtr, "Bpk": bpk, "A2": a2})
    return in_maps


def kernel(x, A, B):
    from concourse.bass_utils import run_bass_kernel_spmd

    x = np.asarray(x, dtype=np.float32)
    A = np.asarray(A, dtype=np.float32)
    B = np.asarray(B, dtype=np.float32)
    orig_shape = x.shape
    x2d = x.reshape(-1, F_IN)
    T_shard = x2d.shape[0] // N_CORES

    key = (T_shard, F_IN, F_OUT, RANK)
    if key not in _CACHE:
        _CACHE[key] = _build_nc(T_shard, F_IN, F_OUT, RANK)
    nc = _CACHE[key]

    in_maps = _pack_inputs(x2d, A, B, T_shard, F_IN, RANK)
    res = run_bass_kernel_spmd(nc, in_maps, core_ids=list(range(N_CORES)))
    out = np.concatenate(
        [np.asarray(r["out"], dtype=np.float32) for r in res.results], axis=0
    )
    return out.reshape(*orig_shape[:-1], F_OUT)
